# revision 11
# baseline (speedup 1.0000x reference)
"""Grouped-GEMM (MoE expert FFN) kernel for 8 Trainium2 NeuronCores.

Problem: x [16384, 2048] f32, weights [8, 8192, 2048] f32, m_splits [8] i64.
Output: concat_e( x[offs[e]:offs[e+1]] @ weights[e].T ) -> [16384, 8192] f32.

Sharding: column-parallel over the output dim. Each of the 8 cores computes
ALL tokens for a 1024-wide slice of D_OUT, for every expert. Per-core work is
identical regardless of m_splits (perfect balance), the SPMD program is
uniform across cores, and no collectives are needed.

Device kernel (per core): for each expert, cache its [2048, 1024] weight
slice (K-major, bf16) in SBUF; stream 512-token tiles of x^T (K-major, bf16);
accumulate 16 K-tiles into two PSUM banks per 128-token row block; copy to
SBUF and DMA the [128, 1024] f32 result block out.

Measured: 903 us max-core HW exec (94% MFU; bf16 PE streaming floor for the
per-core 16384x1024x2048 GEMM is 874 us), absmax-rel error 2.1e-3 vs fp32.

Host side: cast/transpose x and weights (not part of HW exec time), scatter
to the 8 cores, run via run_bass_kernel_spmd, hstack the column slices.
"""

import os
import numpy as np
import ml_dtypes

E = 8
D_IN = 2048
D_OUT = 8192
T = 16384
N_CORES = 8
NPC = D_OUT // N_CORES  # 1024 output columns per core
P = 128
KT = D_IN // P  # 16 k-tiles
M_SUPER = 512  # tokens per x DMA batch
N_HALF = 512  # PSUM bank width (f32)

# Compute dtype for the matmul operands (PSUM accumulation is always fp32).
# HW-measured on this problem (8 cores, max-core exec / absmax-rel error):
#   bf16  903 us  2.1e-3
#   fp16 1082 us  2.4e-4  (sustained fp16 draws more PE power -> P0 downclock
#                          to 2.0 GHz; mini-bursts run at full 2.4 GHz)
#   f32r  ~+10%   1.5e-4  (1-core probe; fp32-size DMA/SBUF footprint)
#   f32   ~3.6x   3.0e-7
# bf16 is the default: this suite's tolerances accommodate bf16 matmul, and
# the kernel is PE-bound so the bf16 streaming rate is the roofline.
COMPUTE_DT = os.environ.get("KERNEL_DTYPE", "bf16")

# Output dtype written by the device kernel. bf16 halves the y HBM write
# traffic (64->32 MB/core) and the PSUM->SBUF copy bytes; the host upcasts
# to f32. Adds <=2^-9 relative rounding on top of the bf16 matmul error.
# Lower HBM/DVE power also helps stay out of the P0 downclock state.
OUT_DT = os.environ.get("KERNEL_OUT_DTYPE", "bf16")

# Number of dummy PE-warmup matmuls (0 = none; first real matmuls run cold).
# 14 x 256-col MMs ~= 3.0us at the cold 1.2 GHz clock: covers the HAM
# activity window and the first (w, x) DMA landing, measured optimum.
WARMUP_MMS = int(os.environ.get("KERNEL_WARMUP_MMS", "14"))

_cache: dict = {}


def _segments(m_splits) -> tuple:
    """Clamped (start, size) per expert, replicating the reference's
    x[offs[e]:offs[e+1]] numpy slice semantics."""
    sizes = [int(s) for s in np.asarray(m_splits)]
    segs = []
    off = 0
    for s in sizes:
        start = min(max(off, 0), T)
        stop = min(max(off + s, 0), T)
        segs.append((start, max(0, stop - start)))
        off += s
    return tuple(segs)


def _build(segments):
    from concourse import bacc
    import concourse.mybir as mybir
    import concourse.tile as tile

    dt = mybir.dt
    cdt = {"fp16": dt.float16, "bf16": dt.bfloat16, "f32r": dt.float32r, "f32": dt.float32}[COMPUTE_DT]
    odt = dt.bfloat16 if OUT_DT == "bf16" else dt.float32
    t_out = sum(m for _, m in segments)
    nc = bacc.Bacc("TRN2", target_bir_lowering=False)
    xT = nc.dram_tensor("xT", [D_IN, T], cdt, kind="ExternalInput")
    wT = nc.dram_tensor("wT", [E, D_IN, NPC], cdt, kind="ExternalInput")
    y = nc.dram_tensor("y", [t_out, NPC], odt, kind="ExternalOutput")

    with tile.TileContext(nc) as tc:
        with (
            tc.tile_pool(name="wp", bufs=2) as wp,
            tc.tile_pool(name="xp", bufs=4) as xp,
            tc.tile_pool(name="op", bufs=8) as op,
            tc.tile_pool(name="pp", bufs=7, space="PSUM") as pp,
        ):
            # Optional PE warmup: dummy matmuls that depend on no DMA, issued
            # while the first (w, x) tiles stream in, to ride out the HAM
            # cold window (K=4/8) before real matmuls. Off by default: the
            # first ~8 real matmuls ride the cold window doing useful work,
            # which nets ~3us over idling through a dummy chain.
            if WARMUP_MMS:
                wu = min(256, N_HALF)
                warm = xp.tile([P, P + wu], cdt, tag="warm", bufs=1)
                nc.vector.memset(warm[:], 0.0)
                pw = pp.tile([P, N_HALF], dt.float32, tag="warm", bufs=1)
                for _ in range(WARMUP_MMS):
                    nc.tensor.matmul(
                        pw[:, :wu],
                        lhsT=warm[:, :P],
                        rhs=warm[:, P : P + wu],
                        start=True,
                        stop=True,
                    )

            out_row = 0
            for e, (start, me) in enumerate(segments):
                if me == 0:
                    continue
                wt = wp.tile([P, KT * NPC], cdt, tag="w")
                first = out_row == 0
                if not first:
                    # later experts: W prefetches during the previous
                    # expert's compute (wp is double-buffered)
                    for k in range(KT):
                        nc.sync.dma_start(
                            wt[:, k * NPC : (k + 1) * NPC],
                            wT[e, k * P : (k + 1) * P, :],
                        )
                for m0 in range(0, me, M_SUPER):
                    msz = min(M_SUPER, me - m0)
                    xt = xp.tile([P, KT * M_SUPER], cdt, tag="x")
                    head = first and m0 == 0
                    for k in range(KT):
                        # very first expert: interleave its weight load with
                        # the first x batch so the k-th (w, x) pair lands
                        # together and the PE's k=0 matmul starts ASAP. The
                        # x slices go out on the scalar ring (idle until the
                        # first output DMA ~19us in) so both input streams
                        # issue in parallel and the k-slices land ~2x sooner.
                        if head:
                            nc.sync.dma_start(
                                wt[:, k * NPC : (k + 1) * NPC],
                                wT[e, k * P : (k + 1) * P, :],
                            )
                        (nc.scalar if head else nc.sync).dma_start(
                            xt[:, k * M_SUPER : k * M_SUPER + msz],
                            xT[k * P : (k + 1) * P, start + m0 : start + m0 + msz],
                        )
                    for ms in range(0, msz, P):
                        mm = min(P, msz - ms)
                        p0 = pp.tile([P, N_HALF], dt.float32, tag="ps")
                        p1 = pp.tile([P, N_HALF], dt.float32, tag="ps")
                        for k in range(KT):
                            lhs = xt[:, k * M_SUPER + ms : k * M_SUPER + ms + mm]
                            nc.tensor.matmul(
                                p0[:mm, :],
                                lhsT=lhs,
                                rhs=wt[:, k * NPC : k * NPC + N_HALF],
                                start=(k == 0),
                                stop=(k == KT - 1),
                            )
                            nc.tensor.matmul(
                                p1[:mm, :],
                                lhsT=lhs,
                                rhs=wt[:, k * NPC + N_HALF : (k + 1) * NPC],
                                start=(k == 0),
                                stop=(k == KT - 1),
                            )
                        # separate half tiles + per-half DMA on the scalar
                        # engine's HWDGE ring: output stream stays off the
                        # input (sync) ring, and the tail's critical path is
                        # one [mm, 512] copy + one short DMA instead of two
                        # copies + a full-width DMA.
                        r = out_row + m0 + ms
                        o0 = op.tile([P, N_HALF], odt, tag="o")
                        nc.vector.tensor_copy(o0[:mm, :], p0[:mm, :])
                        nc.scalar.dma_start(y[r : r + mm, :N_HALF], o0[:mm, :])
                        o1 = op.tile([P, N_HALF], odt, tag="o")
                        nc.vector.tensor_copy(o1[:mm, :], p1[:mm, :])
                        nc.scalar.dma_start(y[r : r + mm, N_HALF:], o1[:mm, :])
                out_row += me
    nc.compile()
    return nc, t_out


last_exec_time_ns = None
last_trace_dir = None


def _install_prof_shim():
    """Register the NTFF profile hook that this image's antenv lacks, so
    run_bass_kernel_spmd(trace=True) can capture HW exec time under axon."""
    import sys
    import types
    import concourse.bass_utils as bass_utils

    try:
        import antenv.axon_hooks  # noqa: F401

        return
    except ImportError:
        pass
    from trn_agent_boot.trn_boot import _ntff_profile_via_ctypes

    hook = _ntff_profile_via_ctypes("/opt/axon/libaxon_pjrt.so")
    mod = types.ModuleType("antenv.axon_hooks")
    mod.get_axon_ntff_profile_hook = lambda: hook
    mod.set_axon_ntff_profile_hook = lambda h: None
    sys.modules["antenv.axon_hooks"] = mod
    import antenv

    antenv.axon_hooks = mod
    bass_utils.upload_artifacts = lambda tmpdir: f"local://{tmpdir}"


def kernel(x: np.ndarray, weights: np.ndarray, m_splits: np.ndarray) -> np.ndarray:
    global last_exec_time_ns, last_trace_dir
    from concourse.bass_utils import run_bass_kernel_spmd

    x = np.asarray(x, dtype=np.float32)
    weights = np.asarray(weights, dtype=np.float32)
    segments = _segments(m_splits)
    if sum(m for _, m in segments) == 0:
        return np.zeros((0, D_OUT), dtype=np.float32)
    if segments not in _cache:
        _cache[segments] = _build(segments)
    nc, t_out = _cache[segments]

    np_dt = {"fp16": np.float16, "bf16": ml_dtypes.bfloat16, "f32r": np.float32, "f32": np.float32}[COMPUTE_DT]
    xT_bf = np.ascontiguousarray(x.T).astype(np_dt)
    w_bf = weights.astype(np_dt)  # [E, D_OUT, D_IN]
    in_maps = []
    for c in range(N_CORES):
        # [E, D_IN, NPC] slice: wT_c[e, k, j] = weights[e, c*NPC + j, k]
        wc = np.ascontiguousarray(
            w_bf[:, c * NPC : (c + 1) * NPC, :].transpose(0, 2, 1)
        )
        in_maps.append({"xT": xT_bf, "wT": wc})

    kwargs = {}
    if os.environ.get("KERNEL_PROFILE"):
        _install_prof_shim()
        tmpdir = os.environ.get("KERNEL_PROFILE_DIR") or None
        if tmpdir:
            # stale NTFFs from a previous profiled run break gauge's
            # ntff->json conversion; start from a clean dir
            import shutil

            shutil.rmtree(tmpdir, ignore_errors=True)
            os.makedirs(tmpdir, exist_ok=True)
        kwargs = dict(trace=True, tmpdir=tmpdir)

    res = run_bass_kernel_spmd(nc, in_maps, core_ids=list(range(N_CORES)), **kwargs)
    last_exec_time_ns = res.exec_time_ns
    if res.instructions_and_trace:
        last_trace_dir = res.instructions_and_trace[1]
    out = np.empty((t_out, D_OUT), dtype=np.float32)
    for c in range(N_CORES):
        out[:, c * NPC : (c + 1) * NPC] = res.results[c]["y"].astype(np.float32)
    return out



# revision 15
# speedup vs baseline: 1.0064x; 1.0064x over previous
"""Grouped-GEMM (MoE expert FFN) kernel for 8 Trainium2 NeuronCores.

Problem: x [16384, 2048] f32, weights [8, 8192, 2048] f32, m_splits [8] i64.
Output: concat_e( x[offs[e]:offs[e+1]] @ weights[e].T ) -> [16384, 8192] f32.

Sharding: column-parallel over the output dim. Each of the 8 cores computes
ALL tokens for a 1024-wide slice of D_OUT, for every expert. Per-core work is
identical regardless of m_splits (perfect balance), the SPMD program is
uniform across cores, and no collectives are needed.

Device kernel (per core): for each expert, cache its [2048, 1024] weight
slice (K-major, bf16) in SBUF; stream 512-token tiles of x^T (K-major, bf16);
accumulate 16 K-tiles into two PSUM banks per 128-token row block; cast each
[128, 512] PSUM half to bf16 on the vector engine and DMA it out on the
scalar engine's HWDGE ring (outputs off the input ring); host upcasts to f32.

Measured (2.4 GHz chip state): 902.1 us max-core HW exec vs 904-905 us for
the f32-out baseline; absmax-rel error 3.5e-3. Per-core roofline is 884 us
(4096 matmuls x (512cy/2.4GHz + 2.5ns NX)) + ~7.4 us framework preamble +
~4 us warmup/ramp + ~3.6 us tail; the matmul stream itself is gap-free.
NOTE: this machine's PE clock flips between 2.4 and 2.0 GHz (P0 power state)
with ambient chip state — the same kernel measures ~902 us or ~1082 us.
bf16 output also halves y write traffic (power margin against P0).
Variants measured worse: no warmup 903.9 us (DMA-gated cold start), warmup
14 + head x-DMAs on the scalar ring 910.2 us (HAM oscillation).

Host side: cast/transpose x and weights (not part of HW exec time), scatter
to the 8 cores, run via run_bass_kernel_spmd, hstack the column slices.
"""

import os
import numpy as np
import ml_dtypes

E = 8
D_IN = 2048
D_OUT = 8192
T = 16384
N_CORES = 8
NPC = D_OUT // N_CORES  # 1024 output columns per core
P = 128
KT = D_IN // P  # 16 k-tiles
M_SUPER = 512  # tokens per x DMA batch
N_HALF = 512  # PSUM bank width (f32)

# Compute dtype for the matmul operands (PSUM accumulation is always fp32).
# HW-measured on this problem (8 cores, max-core exec / absmax-rel error):
#   bf16  903 us  2.1e-3
#   fp16 1082 us  2.4e-4  (sustained fp16 draws more PE power -> P0 downclock
#                          to 2.0 GHz; mini-bursts run at full 2.4 GHz)
#   f32r  ~+10%   1.5e-4  (1-core probe; fp32-size DMA/SBUF footprint)
#   f32   ~3.6x   3.0e-7
# bf16 is the default: this suite's tolerances accommodate bf16 matmul, and
# the kernel is PE-bound so the bf16 streaming rate is the roofline.
COMPUTE_DT = os.environ.get("KERNEL_DTYPE", "bf16")

# Output dtype written by the device kernel. bf16 halves the y HBM write
# traffic (64->32 MB/core) and the PSUM->SBUF copy bytes; the host upcasts
# to f32. Adds <=2^-9 relative rounding on top of the bf16 matmul error.
# Lower HBM/DVE power also helps stay out of the P0 downclock state.
OUT_DT = os.environ.get("KERNEL_OUT_DTYPE", "bf16")

# Number of dummy PE-warmup matmuls (0 = none; first real matmuls run cold).
# 16 x 256-col MMs ~= 3.4us at the cold 1.2 GHz clock: covers the HAM
# activity window and the first (w, x) DMA landing. Measured: 16 -> 902.1us,
# 0 -> 903.9us (DMA-gated cold start), 14 + split-ring head -> 910.2us.
WARMUP_MMS = int(os.environ.get("KERNEL_WARMUP_MMS", "16"))

_cache: dict = {}


def _segments(m_splits) -> tuple:
    """Clamped (start, size) per expert, replicating the reference's
    x[offs[e]:offs[e+1]] numpy slice semantics."""
    sizes = [int(s) for s in np.asarray(m_splits)]
    segs = []
    off = 0
    for s in sizes:
        start = min(max(off, 0), T)
        stop = min(max(off + s, 0), T)
        segs.append((start, max(0, stop - start)))
        off += s
    return tuple(segs)


def _build(segments):
    from concourse import bacc
    import concourse.mybir as mybir
    import concourse.tile as tile

    dt = mybir.dt
    cdt = {"fp16": dt.float16, "bf16": dt.bfloat16, "f32r": dt.float32r, "f32": dt.float32}[COMPUTE_DT]
    odt = dt.bfloat16 if OUT_DT == "bf16" else dt.float32
    t_out = sum(m for _, m in segments)
    nc = bacc.Bacc("TRN2", target_bir_lowering=False)
    xT = nc.dram_tensor("xT", [D_IN, T], cdt, kind="ExternalInput")
    wT = nc.dram_tensor("wT", [E, D_IN, NPC], cdt, kind="ExternalInput")
    y = nc.dram_tensor("y", [t_out, NPC], odt, kind="ExternalOutput")

    with tile.TileContext(nc) as tc:
        with (
            tc.tile_pool(name="wp", bufs=2) as wp,
            tc.tile_pool(name="xp", bufs=4) as xp,
            tc.tile_pool(name="op", bufs=8) as op,
            tc.tile_pool(name="pp", bufs=7, space="PSUM") as pp,
        ):
            # PE warmup: dummy matmuls that depend on no DMA, issued while
            # the first (w, x) tiles stream in. Rides out the HAM cold
            # window (K=4/8) AND the ~4us DMA head so real matmuls start
            # warm; measured better than starting real matmuls cold.
            if WARMUP_MMS:
                wu = min(256, N_HALF)
                warm = xp.tile([P, P + wu], cdt, tag="warm", bufs=1)
                nc.vector.memset(warm[:], 0.0)
                pw = pp.tile([P, N_HALF], dt.float32, tag="warm", bufs=1)
                for _ in range(WARMUP_MMS):
                    nc.tensor.matmul(
                        pw[:, :wu],
                        lhsT=warm[:, :P],
                        rhs=warm[:, P : P + wu],
                        start=True,
                        stop=True,
                    )

            out_row = 0
            for e, (start, me) in enumerate(segments):
                if me == 0:
                    continue
                wt = wp.tile([P, KT * NPC], cdt, tag="w")
                first = out_row == 0
                if not first:
                    # later experts: W prefetches during the previous
                    # expert's compute (wp is double-buffered)
                    for k in range(KT):
                        nc.sync.dma_start(
                            wt[:, k * NPC : (k + 1) * NPC],
                            wT[e, k * P : (k + 1) * P, :],
                        )
                for m0 in range(0, me, M_SUPER):
                    msz = min(M_SUPER, me - m0)
                    xt = xp.tile([P, KT * M_SUPER], cdt, tag="x")
                    head = first and m0 == 0
                    for k in range(KT):
                        # very first expert: interleave its weight load with
                        # the first x batch so the k-th (w, x) pair lands
                        # together and the PE's k=0 matmul starts ASAP.
                        # (Tried routing the head x slices via the scalar
                        # ring to parallelize issue: 910us vs 902us — the
                        # split-ring head stalls the PE and oscillates HAM.)
                        if head:
                            nc.sync.dma_start(
                                wt[:, k * NPC : (k + 1) * NPC],
                                wT[e, k * P : (k + 1) * P, :],
                            )
                        nc.sync.dma_start(
                            xt[:, k * M_SUPER : k * M_SUPER + msz],
                            xT[k * P : (k + 1) * P, start + m0 : start + m0 + msz],
                        )
                    for ms in range(0, msz, P):
                        mm = min(P, msz - ms)
                        p0 = pp.tile([P, N_HALF], dt.float32, tag="ps")
                        p1 = pp.tile([P, N_HALF], dt.float32, tag="ps")
                        for k in range(KT):
                            lhs = xt[:, k * M_SUPER + ms : k * M_SUPER + ms + mm]
                            nc.tensor.matmul(
                                p0[:mm, :],
                                lhsT=lhs,
                                rhs=wt[:, k * NPC : k * NPC + N_HALF],
                                start=(k == 0),
                                stop=(k == KT - 1),
                            )
                            nc.tensor.matmul(
                                p1[:mm, :],
                                lhsT=lhs,
                                rhs=wt[:, k * NPC + N_HALF : (k + 1) * NPC],
                                start=(k == 0),
                                stop=(k == KT - 1),
                            )
                        # separate half tiles + per-half DMA on the scalar
                        # engine's HWDGE ring: output stream stays off the
                        # input (sync) ring, and the tail's critical path is
                        # one [mm, 512] copy + one short DMA instead of two
                        # copies + a full-width DMA.
                        r = out_row + m0 + ms
                        o0 = op.tile([P, N_HALF], odt, tag="o")
                        nc.vector.tensor_copy(o0[:mm, :], p0[:mm, :])
                        nc.scalar.dma_start(y[r : r + mm, :N_HALF], o0[:mm, :])
                        o1 = op.tile([P, N_HALF], odt, tag="o")
                        nc.vector.tensor_copy(o1[:mm, :], p1[:mm, :])
                        nc.scalar.dma_start(y[r : r + mm, N_HALF:], o1[:mm, :])
                out_row += me
    nc.compile()
    return nc, t_out


last_exec_time_ns = None
last_trace_dir = None


def _install_prof_shim():
    """Register the NTFF profile hook that this image's antenv lacks, so
    run_bass_kernel_spmd(trace=True) can capture HW exec time under axon."""
    import sys
    import types
    import concourse.bass_utils as bass_utils

    try:
        import antenv.axon_hooks  # noqa: F401

        return
    except ImportError:
        pass
    from trn_agent_boot.trn_boot import _ntff_profile_via_ctypes

    hook = _ntff_profile_via_ctypes("/opt/axon/libaxon_pjrt.so")
    mod = types.ModuleType("antenv.axon_hooks")
    mod.get_axon_ntff_profile_hook = lambda: hook
    mod.set_axon_ntff_profile_hook = lambda h: None
    sys.modules["antenv.axon_hooks"] = mod
    import antenv

    antenv.axon_hooks = mod
    bass_utils.upload_artifacts = lambda tmpdir: f"local://{tmpdir}"


def kernel(x: np.ndarray, weights: np.ndarray, m_splits: np.ndarray) -> np.ndarray:
    global last_exec_time_ns, last_trace_dir
    from concourse.bass_utils import run_bass_kernel_spmd

    x = np.asarray(x, dtype=np.float32)
    weights = np.asarray(weights, dtype=np.float32)
    segments = _segments(m_splits)
    if sum(m for _, m in segments) == 0:
        return np.zeros((0, D_OUT), dtype=np.float32)
    if segments not in _cache:
        _cache[segments] = _build(segments)
    nc, t_out = _cache[segments]

    np_dt = {"fp16": np.float16, "bf16": ml_dtypes.bfloat16, "f32r": np.float32, "f32": np.float32}[COMPUTE_DT]
    xT_bf = np.ascontiguousarray(x.T).astype(np_dt)
    w_bf = weights.astype(np_dt)  # [E, D_OUT, D_IN]
    in_maps = []
    for c in range(N_CORES):
        # [E, D_IN, NPC] slice: wT_c[e, k, j] = weights[e, c*NPC + j, k]
        wc = np.ascontiguousarray(
            w_bf[:, c * NPC : (c + 1) * NPC, :].transpose(0, 2, 1)
        )
        in_maps.append({"xT": xT_bf, "wT": wc})

    kwargs = {}
    if os.environ.get("KERNEL_PROFILE"):
        _install_prof_shim()
        tmpdir = os.environ.get("KERNEL_PROFILE_DIR") or None
        if tmpdir:
            # stale NTFFs from a previous profiled run break gauge's
            # ntff->json conversion; start from a clean dir
            import shutil

            shutil.rmtree(tmpdir, ignore_errors=True)
            os.makedirs(tmpdir, exist_ok=True)
        kwargs = dict(trace=True, tmpdir=tmpdir)

    res = run_bass_kernel_spmd(nc, in_maps, core_ids=list(range(N_CORES)), **kwargs)
    last_exec_time_ns = res.exec_time_ns
    if res.instructions_and_trace:
        last_trace_dir = res.instructions_and_trace[1]
    out = np.empty((t_out, D_OUT), dtype=np.float32)
    for c in range(N_CORES):
        out[:, c * NPC : (c + 1) * NPC] = res.results[c]["y"].astype(np.float32)
    return out

